# revision 1
# baseline (speedup 1.0000x reference)
"""MoE (top-2 of 8 experts, gelu MLP) on 8 TRN2 NeuronCores.

Strategy (expert-parallel, per the sharding hint):
  Phase A (device, data-parallel over tokens): router scores = x @ router_w.T
    in fp32, top-2 via the DVE max8 instruction, softmax-over-2 via the exact
    sigmoid identity. Outputs per-token per-expert combine weights w[T, E]
    (0 for unselected experts).
  Host dispatch ("all-to-all"): gather each expert's selected token columns
    (from the device-computed weights) into a per-core capacity-padded batch;
    this is the sharding step for phase B.
  Phase B (device, expert-parallel): each core runs one expert's FFN
    out = gelu(xsel @ w1[e].T) @ w2[e].T * w[:, None] over its gathered
    tokens, with float32r (TF32-like, ~1e-4) matmuls at full PE rate.
  Host combine: scatter-add the per-expert results back (each token appears
    in exactly two experts' batches).

kernel(**inputs) -> np.ndarray  takes FULL inputs, returns FULL output.
"""

import numpy as np

import concourse.bass as bass
import concourse.mybir as mybir
from concourse import bacc
from concourse.tile import TileContext
from concourse.bass_utils import run_bass_kernel_spmd

HIDDEN = 1024
NUM_EXPERTS = 8
TOP_K = 2
FFN = 4096
BATCH, SEQ = 4, 2048
T = BATCH * SEQ          # 8192 tokens
NCORES = 8
TPC = T // NCORES        # tokens per core in phase A
P = 128
DK = HIDDEN // P         # 8 contraction tiles over hidden
FQ = 8                   # F blocks in phase B
FQ_SIZE = FFN // FQ      # 512
TT = 256                 # phase-B token tile (fits mm2 psum in 4 banks)

f32 = mybir.dt.float32
f32r = mybir.dt.float32r


def _round_f32r(a: np.ndarray) -> np.ndarray:
    """Round-to-nearest-even to float32r (11 explicit mantissa bits), matching
    the TRN2 DVE fp32->fp32r rounding bit-exactly."""
    u = np.ascontiguousarray(a, dtype=np.float32).view(np.uint32)
    low = u & np.uint32(0xFFF)
    base = u & ~np.uint32(0xFFF)
    up = base + np.uint32(0x1000)
    lsb = (base >> np.uint32(12)) & np.uint32(1)
    round_up = (low > 0x800) | ((low == 0x800) & (lsb == 1))
    return np.where(round_up, up, base).view(np.float32)


def _build_phase_a(repeat=False):
    """Per core: scores for TPC tokens (fp32 matmul) -> top-2 -> weights.

    inputs:  xt [HIDDEN, TPC] fp32 (column shard of x.T), rt [HIDDEN, E] fp32
    output:  w  [TPC, E] fp32 combine weights (0 where expert unselected)
    """
    nc = bacc.Bacc(None)
    xt_d = nc.declare_dram_parameter("xt", [HIDDEN, TPC], f32, isOutput=False)
    rt_d = nc.declare_dram_parameter("rt", [HIDDEN, NUM_EXPERTS], f32, isOutput=False)
    w_d = nc.declare_dram_parameter("w", [TPC, NUM_EXPERTS], f32, isOutput=True)
    if repeat:
        r_d = nc.declare_dram_parameter("r", [1, 1], mybir.dt.uint32, isOutput=False)

    MT = TPC // P  # token tiles per core
    from contextlib import ExitStack
    with TileContext(nc) as tc, ExitStack() as stk:
        if repeat:
            rp = stk.enter_context(tc.tile_pool(name="rp", bufs=1))
            r_t = rp.tile([1, 1], mybir.dt.uint32)
            nc.sync.dma_start(out=r_t[:], in_=r_d[:])
            _, (r_val,) = nc.values_load_multi_w_load_instructions(
                r_t[:], min_val=1, max_val=1 << 24)
            stk.enter_context(tc.For_i(0, r_val))
        with tc.tile_pool(name="sb", bufs=1) as pool, \
             tc.tile_pool(name="work", bufs=3) as wp, \
             tc.tile_pool(name="ps", bufs=2, space="PSUM") as psp:
            xt_t = pool.tile([P, DK, TPC], f32)
            rt_t = pool.tile([P, DK, NUM_EXPERTS], f32)
            nc.sync.dma_start(out=rt_t[:], in_=rt_d[:].rearrange("(ko ki) e -> ki ko e", ki=P))
            xt_r = xt_d[:].rearrange("(ko ki) t -> ki ko t", ki=P)
            for m in range(MT):
                nc.sync.dma_start(out=xt_t[:, :, m * P:(m + 1) * P],
                                  in_=xt_r[:, :, m * P:(m + 1) * P])

            for m in range(MT):
                ps = psp.tile([P, NUM_EXPERTS], f32)
                for k in range(DK):
                    nc.tensor.matmul(ps[:], xt_t[:, k, m * P:(m + 1) * P], rt_t[:, k],
                                     start=(k == 0), stop=(k == DK - 1))
                s_t = wp.tile([P, NUM_EXPERTS], f32, tag="s")
                nc.vector.tensor_copy(s_t[:], ps[:])
                v8 = wp.tile([P, 8], f32, tag="v8")
                nc.vector.max(out=v8[:], in_=s_t[:])
                d12 = wp.tile([P, 1], f32, tag="d12")
                nc.vector.tensor_sub(d12[:], v8[:, 0:1], v8[:, 1:2])
                sg1 = wp.tile([P, 1], f32, tag="sg1")
                sg2 = wp.tile([P, 1], f32, tag="sg2")
                nc.scalar.activation(sg1[:], d12[:], mybir.ActivationFunctionType.Sigmoid)
                nc.scalar.activation(sg2[:], d12[:], mybir.ActivationFunctionType.Sigmoid,
                                     scale=-1.0)
                m1 = wp.tile([P, NUM_EXPERTS], f32, tag="m1")
                m2 = wp.tile([P, NUM_EXPERTS], f32, tag="m2")
                nc.vector.tensor_tensor(out=m1[:], in0=s_t[:],
                                        in1=v8[:, 0:1].to_broadcast([P, NUM_EXPERTS]),
                                        op=mybir.AluOpType.is_equal)
                nc.vector.tensor_tensor(out=m2[:], in0=s_t[:],
                                        in1=v8[:, 1:2].to_broadcast([P, NUM_EXPERTS]),
                                        op=mybir.AluOpType.is_equal)
                w_t = wp.tile([P, NUM_EXPERTS], f32, tag="wt")
                nc.vector.tensor_mul(m1[:], m1[:], sg1[:].to_broadcast([P, NUM_EXPERTS]))
                nc.vector.tensor_mul(m2[:], m2[:], sg2[:].to_broadcast([P, NUM_EXPERTS]))
                nc.vector.tensor_add(w_t[:], m1[:], m2[:])
                nc.sync.dma_start(out=w_d[m * P:(m + 1) * P, :], in_=w_t[:])
    nc.compile()
    return nc


def _build_phase_b(C: int, act=None, repeat=False):
    """Per core: one expert's FFN over C gathered tokens (float32r matmuls).

    inputs: xt  [HIDDEN, C] f32r   gathered+transposed tokens (pre-rounded)
            w1t [HIDDEN, FFN] f32r  expert w1 transposed (pre-rounded)
            w2t [FFN, HIDDEN] f32r  expert w2 transposed (pre-rounded)
            ws  [P, C//P] f32       per-token combine weight, ws[p, m] = w[m*P+p]
    output: o   [C, HIDDEN] f32     w * (gelu(x @ w1.T) @ w2.T)
    """
    assert C % TT == 0
    if act is None:
        act = mybir.ActivationFunctionType.Gelu
    nc = bacc.Bacc(None)
    xt_d = nc.declare_dram_parameter("xt", [HIDDEN, C], f32r, isOutput=False)
    w1t_d = nc.declare_dram_parameter("w1t", [HIDDEN, FFN], f32r, isOutput=False)
    w2t_d = nc.declare_dram_parameter("w2t", [FFN, HIDDEN], f32r, isOutput=False)
    ws_d = nc.declare_dram_parameter("ws", [P, C // P], f32, isOutput=False)
    if repeat:
        r_d = nc.declare_dram_parameter("r", [1, 1], mybir.dt.uint32, isOutput=False)
    o_d = nc.declare_dram_parameter("o", [C, HIDDEN], f32, isOutput=True)

    # token tiles: 512-wide (N=512 moving ops amortize LDWEIGHTS), 256 tail ok
    assert C % 256 == 0
    t_sizes = [512] * (C // 512) + ([256] if C % 512 else [])
    t_offs = np.cumsum([0] + t_sizes)[:-1].tolist()
    FM = FQ_SIZE // P      # f-subtiles per block
    DN = HIDDEN // 512     # 2 output column blocks

    from contextlib import ExitStack
    with TileContext(nc) as tc, ExitStack() as stk:
        if repeat:
            rp = stk.enter_context(tc.tile_pool(name="rp", bufs=1))
            r_t = rp.tile([1, 1], mybir.dt.uint32)
            nc.sync.dma_start(out=r_t[:], in_=r_d[:])
            _, (r_val,) = nc.values_load_multi_w_load_instructions(
                r_t[:], min_val=1, max_val=1 << 24)
            stk.enter_context(tc.For_i(0, r_val))
        with tc.tile_pool(name="wq", bufs=2) as wqp, \
             tc.tile_pool(name="acc", bufs=1) as accp, \
             tc.tile_pool(name="xs", bufs=2) as xsp, \
             tc.tile_pool(name="h", bufs=2) as hp, \
             tc.tile_pool(name="fin", bufs=2) as finp, \
             tc.tile_pool(name="ps1", bufs=3, space="PSUM") as ps1p, \
             tc.tile_pool(name="ps2", bufs=3, space="PSUM") as ps2p:
            acc = accp.tile([P, C // P, HIDDEN], f32)
            ws_t = accp.tile([P, C // P], f32)
            nc.sync.dma_start(out=ws_t[:], in_=ws_d[:])

            for q in range(FQ):
                w1q = wqp.tile([P, DK, FQ_SIZE], f32r, tag="w1q")
                w2q = wqp.tile([P, FM, HIDDEN], f32r, tag="w2q")
                w1r = w1t_d[:, q * FQ_SIZE:(q + 1) * FQ_SIZE].rearrange(
                    "(ko ki) f -> ki ko f", ki=P)
                for k in range(DK):
                    nc.sync.dma_start(out=w1q[:, k], in_=w1r[:, k])
                w2r = w2t_d[q * FQ_SIZE:(q + 1) * FQ_SIZE, :].rearrange(
                    "(fo fi) d -> fi fo d", fi=P)
                for fk in range(FM):
                    nc.sync.dma_start(out=w2q[:, fk], in_=w2r[:, fk])

                for t_off, t_size in zip(t_offs, t_sizes):
                    x_t = xsp.tile([P, DK, 512], f32r, tag="xt", name="x_t")[:, :, :t_size]
                    x_r = xt_d[:, t_off:t_off + t_size].rearrange(
                        "(ko ki) c -> ki ko c", ki=P)
                    if q == 0 and t_off == 0:
                        # per-k chunks so the first matmul starts ~15us sooner
                        for k in range(DK):
                            nc.sync.dma_start(out=x_t[:, k], in_=x_r[:, k])
                    else:
                        nc.sync.dma_start(out=x_t[:], in_=x_r)
                    h_t = hp.tile([P, FM, 512], f32r, tag="h", name="h_t")[:, :, :t_size]
                    for fm in range(FM):
                        ph = ps1p.tile([P, 512], f32, tag="ph", name="ph")[:, :t_size]
                        for k in range(DK):
                            nc.tensor.matmul(ph[:], w1q[:, k, fm * P:(fm + 1) * P],
                                             x_t[:, k], start=(k == 0), stop=(k == DK - 1))
                        nc.scalar.activation(h_t[:, fm], ph[:], act)
                    for tm in range(t_size // P):
                        mt = t_off // P + tm
                        for dn in range(DN):
                            po = ps2p.tile([P, 512], f32, tag="po")
                            for fk in range(FM):
                                nc.tensor.matmul(po[:],
                                                 h_t[:, fk, tm * P:(tm + 1) * P],
                                                 w2q[:, fk, dn * 512:(dn + 1) * 512],
                                                 start=(fk == 0), stop=(fk == FM - 1))
                            if q == 0:
                                nc.vector.tensor_copy(
                                    acc[:, mt, dn * 512:(dn + 1) * 512], po[:])
                            else:
                                nc.vector.tensor_add(
                                    acc[:, mt, dn * 512:(dn + 1) * 512],
                                    acc[:, mt, dn * 512:(dn + 1) * 512], po[:])

            o_r = o_d[:].rearrange("(mo p) d -> p mo d", p=P)
            for mt in range(C // P):
                fin = finp.tile([P, HIDDEN], f32, tag="fin")
                nc.vector.tensor_scalar_mul(fin[:], acc[:, mt], ws_t[:, mt:mt + 1])
                nc.sync.dma_start(out=o_r[:, mt], in_=fin[:])
    nc.compile()
    return nc


_A_CACHE = {}
_B_CACHE = {}
LAST_HW_NS = None


def _run_spmd(nc, in_maps, retries=2):
    """run_bass_kernel_spmd with retry: device crashes on this axon path are
    occasionally transient (NRT_EXEC_UNIT_UNRECOVERABLE recovers on a fresh
    attempt)."""
    last = None
    for attempt in range(retries + 1):
        try:
            return run_bass_kernel_spmd(nc, in_maps, list(range(NCORES)))
        except Exception as e:  # noqa: BLE001
            last = e
            import time as _time
            _time.sleep(2.0 * (attempt + 1))
    raise last


def _phase_a_nc():
    if "a" not in _A_CACHE:
        _A_CACHE["a"] = _build_phase_a()
    return _A_CACHE["a"]


def _phase_b_nc(C):
    if C not in _B_CACHE:
        _B_CACHE[C] = _build_phase_b(C)
    return _B_CACHE[C]


def kernel(x, router_w, expert_w1, expert_w2):
    xf = np.ascontiguousarray(x.reshape(T, HIDDEN), dtype=np.float32)
    xT = np.ascontiguousarray(xf.T)                       # [D, T]
    rT = np.ascontiguousarray(router_w.T.astype(np.float32))  # [D, E]

    # ---- phase A: router on device (data-parallel over tokens) ----
    nc_a = _phase_a_nc()
    in_a = [{"xt": np.ascontiguousarray(xT[:, i * TPC:(i + 1) * TPC]), "rt": rT}
            for i in range(NCORES)]
    res_a = _run_spmd(nc_a, in_a)
    w_all = np.concatenate([res_a.results[i]["w"] for i in range(NCORES)], axis=0)

    # ---- host dispatch: gather each expert's tokens (device-decided) ----
    idx = [np.nonzero(w_all[:, e] > 0.0)[0] for e in range(NUM_EXPERTS)]
    cmax = max(len(i) for i in idx)
    # SBUF accumulator fits up to 2560 tokens; larger loads run in chunks.
    MAXC = 2560
    C = min(((cmax + TT - 1) // TT) * TT, MAXC)
    n_chunks = (cmax + C - 1) // C
    xT_r = _round_f32r(xT)
    w1t_r = [_round_f32r(expert_w1[e].T) for e in range(NUM_EXPERTS)]
    w2t_r = [_round_f32r(expert_w2[e].T) for e in range(NUM_EXPERTS)]
    nc_b = _phase_b_nc(C)

    out = np.zeros((T, HIDDEN), dtype=np.float32)
    for r in range(n_chunks):
        in_b = []
        for e in range(NUM_EXPERTS):
            ids = idx[e][r * C:(r + 1) * C]
            n = len(ids)
            xsel = np.zeros((HIDDEN, C), dtype=np.float32)
            xsel[:, :n] = xT_r[:, ids]
            wsel = np.zeros(C, dtype=np.float32)
            wsel[:n] = w_all[ids, e]
            in_b.append({
                "xt": xsel,
                "w1t": w1t_r[e],
                "w2t": w2t_r[e],
                "ws": np.ascontiguousarray(wsel.reshape(C // P, P).T),
            })
        # ---- phase B: expert FFN on device (expert-parallel) ----
        res_b = _run_spmd(nc_b, in_b)
        # ---- host combine: scatter-add (indices within an expert unique) ----
        for e in range(NUM_EXPERTS):
            ids = idx[e][r * C:(r + 1) * C]
            out[ids] += res_b.results[e]["o"][:len(ids)]

    # cost-model exec-time estimate (NTFF profiling unavailable on this path)
    global LAST_HW_NS
    try:
        if ("t", C) not in _B_CACHE:
            from concourse.timeline_sim import TimelineSim
            _B_CACHE[("t", C)] = (TimelineSim(nc_a).simulate()
                                  + TimelineSim(nc_b).simulate() * n_chunks)
        LAST_HW_NS = int(_B_CACHE[("t", C)])
    except Exception:  # noqa: BLE001
        pass
    return out.reshape(BATCH, SEQ, HIDDEN)



# revision 15
# speedup vs baseline: 1.0910x; 1.0910x over previous
"""MoE (top-2 of 8 experts, gelu MLP) on 8 TRN2 NeuronCores.

Strategy (expert-parallel, per the sharding hint):
  Phase A (device, data-parallel over tokens): router scores = x @ router_w.T
    in fp32, top-2 via the DVE max8 instruction, softmax-over-2 via the exact
    sigmoid identity. Outputs per-token per-expert combine weights w[T, E]
    (0 for unselected experts).
  Host dispatch ("all-to-all"): gather each expert's selected token columns
    (from the device-computed weights) into a per-core capacity-padded batch;
    this is the sharding step for phase B.
  Phase B (device, expert-parallel): each core runs one expert's FFN
    out = gelu(xsel @ w1[e].T) @ w2[e].T * w[:, None] over its gathered
    tokens. All matmuls in bf16 (full PE rate, ~5e-3 rel err, well under the
    2e-2 gate). Both weight matrices stay resident in SBUF (16.8 MB bf16),
    tokens stream through in 512-wide tiles, and the FFN contraction for
    mm2 accumulates fully in PSUM - no SBUF accumulator, no re-reads of x.
  Host combine: scatter-add the per-expert results back (each token appears
    in exactly two experts' batches).

kernel(**inputs) -> np.ndarray  takes FULL inputs, returns FULL output.
"""

import numpy as np
import ml_dtypes

import concourse.bass as bass
import concourse.mybir as mybir
from concourse import bacc
from concourse.tile import TileContext
from concourse.bass_utils import run_bass_kernel_spmd

HIDDEN = 1024
NUM_EXPERTS = 8
TOP_K = 2
FFN = 4096
BATCH, SEQ = 4, 2048
T = BATCH * SEQ          # 8192 tokens
NCORES = 8
TPC = T // NCORES        # tokens per core in phase A
P = 128
DK = HIDDEN // P         # 8 contraction tiles over hidden
FM = FFN // P            # 32 f-subtiles (128 wide) over the FFN dim
DN = HIDDEN // 512       # 2 output column blocks
MAXC = 8192              # per-launch token capacity (SBUF-independent here)

f32 = mybir.dt.float32
bf16 = mybir.dt.bfloat16
npbf16 = ml_dtypes.bfloat16


def _build_phase_a(repeat=False):
    """Per core: scores for TPC tokens (fp32 matmul) -> top-2 -> weights.

    inputs:  xt [HIDDEN, TPC] fp32 (column shard of x.T), rt [HIDDEN, E] fp32
    output:  w  [TPC, E] fp32 combine weights (0 where expert unselected)
    """
    nc = bacc.Bacc(None)
    xt_d = nc.declare_dram_parameter("xt", [HIDDEN, TPC], f32, isOutput=False)
    rt_d = nc.declare_dram_parameter("rt", [HIDDEN, NUM_EXPERTS], f32, isOutput=False)
    w_d = nc.declare_dram_parameter("w", [TPC, NUM_EXPERTS], f32, isOutput=True)
    if repeat:
        r_d = nc.declare_dram_parameter("r", [1, 1], mybir.dt.uint32, isOutput=False)

    MT = TPC // P  # token tiles per core
    # input DMA chunking (in token tiles): bigger chunks up front for DMA
    # efficiency, single-tile chunks at the end so the last tile's top-2
    # pipeline starts as soon as possible
    chunks = [(0, 2), (2, 2), (4, 2), (6, 1), (7, 1)]
    from contextlib import ExitStack
    with TileContext(nc) as tc, ExitStack() as stk:
        if repeat:
            rp = stk.enter_context(tc.tile_pool(name="rp", bufs=1))
            r_t = rp.tile([1, 1], mybir.dt.uint32)
            nc.sync.dma_start(out=r_t[:], in_=r_d[:])
            _, (r_val,) = nc.values_load_multi_w_load_instructions(
                r_t[:], min_val=1, max_val=1 << 24)
            stk.enter_context(tc.For_i(0, r_val))
        with tc.tile_pool(name="sb", bufs=1) as pool, \
             tc.tile_pool(name="work", bufs=3) as wp, \
             tc.tile_pool(name="ps", bufs=4, space="PSUM") as psp:
            xt_t = pool.tile([P, DK, TPC], f32)
            rt_t = pool.tile([P, DK, NUM_EXPERTS], f32)
            wout = pool.tile([P, MT, NUM_EXPERTS], f32)
            nc.sync.dma_start(out=rt_t[:], in_=rt_d[:].rearrange("(ko ki) e -> ki ko e", ki=P))
            xt_r = xt_d[:].rearrange("(ko ki) t -> ki ko t", ki=P)
            # interleave input chunks across the two HWDGE queues (SP, Act)
            for c, (m, mn) in enumerate(chunks):
                q = nc.sync if c % 2 == 0 else nc.scalar
                q.dma_start(out=xt_t[:, :, m * P:(m + mn) * P],
                            in_=xt_r[:, :, m * P:(m + mn) * P])

            for m in range(MT):
                ps = psp.tile([P, NUM_EXPERTS], f32)
                for k in range(DK):
                    nc.tensor.matmul(ps[:], xt_t[:, k, m * P:(m + 1) * P], rt_t[:, k],
                                     start=(k == 0), stop=(k == DK - 1))
                v8 = wp.tile([P, 8], f32, tag="v8")
                nc.vector.max(out=v8[:], in_=ps[:])
                sg1 = wp.tile([P, 1], f32, tag="sg1")
                sg2 = wp.tile([P, 1], f32, tag="sg2")
                # softmax over top-2 == sigmoid(+/-(v1 - v2)), fused sub via
                # the activation bias/scale path
                nc.scalar.activation(sg1[:], v8[:, 1:2],
                                     mybir.ActivationFunctionType.Sigmoid,
                                     scale=-1.0, bias=v8[:, 0:1])
                nc.scalar.activation(sg2[:], v8[:, 0:1],
                                     mybir.ActivationFunctionType.Sigmoid,
                                     scale=-1.0, bias=v8[:, 1:2])
                m1 = wp.tile([P, NUM_EXPERTS], f32, tag="m1")
                m2 = wp.tile([P, NUM_EXPERTS], f32, tag="m2")
                # w_e = (s==v1)*sg1 + (s==v2)*sg2, two fused compare-mults
                nc.vector.scalar_tensor_tensor(
                    out=m1[:], in0=ps[:], scalar=v8[:, 0:1],
                    in1=sg1[:].to_broadcast([P, NUM_EXPERTS]),
                    op0=mybir.AluOpType.is_equal, op1=mybir.AluOpType.mult)
                nc.vector.scalar_tensor_tensor(
                    out=m2[:], in0=ps[:], scalar=v8[:, 1:2],
                    in1=sg2[:].to_broadcast([P, NUM_EXPERTS]),
                    op0=mybir.AluOpType.is_equal, op1=mybir.AluOpType.mult)
                nc.vector.tensor_add(wout[:, m], m1[:], m2[:])
            # one batched result DMA: shorter tail than 8 small ones
            nc.sync.dma_start(out=w_d[:].rearrange("(mo p) e -> p mo e", p=P),
                              in_=wout[:])
    nc.compile()
    return nc


def _build_phase_b(C: int, act=None, repeat=False):
    """Per core: one expert's FFN over C gathered tokens (bf16 matmuls).

    inputs: xt  [P, DK, C] bf16     gathered tokens, (ki, ko, c) layout
            w1b [FM, P, DK*P] bf16  expert w1, (fm, ki, ko*fi) layout
            w2b [FM, P, HIDDEN] bf16 expert w2.T, (fk, fi, d) layout
            ws  [P, C//P] f32       per-token combine weight, ws[p, m] = w[m*P+p]
    output: o   [C, HIDDEN] bf16    w * (gelu(x @ w1.T) @ w2.T)
    """
    assert C % P == 0
    if act is None:
        act = mybir.ActivationFunctionType.Gelu
    nc = bacc.Bacc(None)
    xt_d = nc.declare_dram_parameter("xt", [P, DK, C], bf16, isOutput=False)
    w1_d = nc.declare_dram_parameter("w1b", [FM, P, DK * P], bf16, isOutput=False)
    w2_d = nc.declare_dram_parameter("w2b", [FM, P, HIDDEN], bf16, isOutput=False)
    ws_d = nc.declare_dram_parameter("ws", [P, C // P], f32, isOutput=False)
    if repeat:
        r_d = nc.declare_dram_parameter("r", [1, 1], mybir.dt.uint32, isOutput=False)
    o_d = nc.declare_dram_parameter("o", [C, HIDDEN], bf16, isOutput=True)

    # Token tiles: a 256-wide head tile starts PE after only ~1.5us of x DMA
    # (its mm2 is deferred past the next tile's mm1 so the short gelu chain
    # never stalls PE), then 512-wide tiles whose 1.7us/block mm1 pace hides
    # the weight stream, then the 128-multiple remainder.
    if C <= 512:
        t_sizes = [C]
    else:
        rem = (C - 256) % 512
        t_sizes = [256] + [512] * ((C - 256) // 512) + ([rem] if rem else [])
    t_offs = np.cumsum([0] + t_sizes)[:-1].tolist()

    from contextlib import ExitStack
    with TileContext(nc) as tc, ExitStack() as stk:
        if repeat:
            rp = stk.enter_context(tc.tile_pool(name="rp", bufs=1))
            r_t = rp.tile([1, 1], mybir.dt.uint32)
            nc.sync.dma_start(out=r_t[:], in_=r_d[:])
            _, (r_val,) = nc.values_load_multi_w_load_instructions(
                r_t[:], min_val=1, max_val=1 << 24)
            stk.enter_context(tc.For_i(0, r_val))
        with tc.tile_pool(name="wt", bufs=1) as wtp, \
             tc.tile_pool(name="xs", bufs=2) as xsp, \
             tc.tile_pool(name="h", bufs=1) as hp, \
             tc.tile_pool(name="ot", bufs=2) as otp, \
             tc.tile_pool(name="wm", bufs=1, space="PSUM") as wmp, \
             tc.tile_pool(name="ps1", bufs=2, space="PSUM") as ps1p, \
             tc.tile_pool(name="ps2", bufs=2, space="PSUM") as ps2p:
            # dependency-free warmup matmuls: ramp the PE to full p-state
            # during the initial DMA window it would otherwise idle through
            scr = wtp.tile([P, 512], bf16, name="scr")
            nc.vector.memset(scr[:], 0.0)
            wm_ps = wmp.tile([P, 512], f32, name="wm_ps")
            for _ in range(WARMUP_MM):
                nc.tensor.matmul(wm_ps[:], scr[:, :P], scr[:], start=True, stop=True)
            w1s = wtp.tile([P, FM, DK * P], bf16)
            w2s = wtp.tile([P, FM, HIDDEN], bf16)
            ws_t = wtp.tile([P, C // P], f32)
            defer0 = len(t_sizes) > 1
            h0 = wtp.tile([P, FM, t_sizes[0]], bf16, name="h0") if defer0 else None

            # x tiles stream on the Activation HWDGE queue; weights on SP's.
            x_ts = []
            for ti, (t_off, t_size) in enumerate(zip(t_offs, t_sizes)):
                if ti < 2:
                    x_t = xsp.tile([P, DK, 512], bf16, tag="x",
                                   name="x_t")[:, :, :t_size]
                    nc.scalar.dma_start(out=x_t[:],
                                        in_=xt_d[:, :, t_off:t_off + t_size])
                    x_ts.append(x_t)
                else:
                    x_ts.append(None)  # allocated in the loop (double-buffered)
            for fm in range(FM):
                nc.sync.dma_start(out=w1s[:, fm], in_=w1_d[fm])
            for fk in range(FM):
                nc.sync.dma_start(out=w2s[:, fk], in_=w2_d[fk])
            nc.sync.dma_start(out=ws_t[:], in_=ws_d[:])

            o_r = o_d[:].rearrange("(mo p) d -> p mo d", p=P)

            def mm1(x_t, h_t, t_size):
                for fm in range(FM):
                    ph = ps1p.tile([P, 512], f32, tag="ph", name="ph")[:, :t_size]
                    for k in range(DK):
                        nc.tensor.matmul(ph[:], w1s[:, fm, k * P:(k + 1) * P],
                                         x_t[:, k], start=(k == 0), stop=(k == DK - 1))
                    nc.scalar.activation(h_t[:, fm], ph[:], act)

            def mm2(h_t, t_off, t_size):
                for tm in range(t_size // P):
                    mt = t_off // P + tm
                    for dn in range(DN):
                        po = ps2p.tile([P, 512], f32, tag="po", name="po")
                        for fk in range(FM):
                            nc.tensor.matmul(po[:],
                                             h_t[:, fk, tm * P:(tm + 1) * P],
                                             w2s[:, fk, dn * 512:(dn + 1) * 512],
                                             start=(fk == 0), stop=(fk == FM - 1))
                        o_t = otp.tile([P, 512], bf16, tag="o", name="o_t")
                        nc.vector.tensor_scalar_mul(o_t[:], po[:], ws_t[:, mt:mt + 1])
                        nc.sync.dma_start(out=o_r[:, mt, dn * 512:(dn + 1) * 512],
                                          in_=o_t[:])

            for ti, (t_off, t_size) in enumerate(zip(t_offs, t_sizes)):
                x_t = x_ts[ti]
                if x_t is None:
                    x_t = xsp.tile([P, DK, 512], bf16, tag="x",
                                   name="x_t")[:, :, :t_size]
                    nc.scalar.dma_start(out=x_t[:],
                                        in_=xt_d[:, :, t_off:t_off + t_size])
                if defer0 and ti == 0:
                    mm1(x_t, h0, t_size)
                    continue
                h_t = hp.tile([P, FM, 512], bf16, tag="h", name="h_t")[:, :, :t_size]
                mm1(x_t, h_t, t_size)
                if defer0 and ti == 1:
                    mm2(h0, 0, t_sizes[0])
                mm2(h_t, t_off, t_size)
    nc.compile()
    return nc


_A_CACHE = {}
_B_CACHE = {}
_W_CACHE = {}
LAST_HW_NS = None


def _run_spmd(nc, in_maps, retries=2):
    """run_bass_kernel_spmd with retry: device crashes on this axon path are
    occasionally transient (NRT_EXEC_UNIT_UNRECOVERABLE recovers on a fresh
    attempt)."""
    last = None
    for attempt in range(retries + 1):
        try:
            return run_bass_kernel_spmd(nc, in_maps, list(range(NCORES)))
        except Exception as e:  # noqa: BLE001
            last = e
            import time as _time
            _time.sleep(2.0 * (attempt + 1))
    raise last


def _phase_a_nc():
    if "a" not in _A_CACHE:
        _A_CACHE["a"] = _build_phase_a()
    return _A_CACHE["a"]


def _phase_b_nc(C):
    if C not in _B_CACHE:
        _B_CACHE[C] = _build_phase_b(C)
    return _B_CACHE[C]


def _expert_weights_bf16(expert_w1, expert_w2):
    """Per-expert bf16 weight blocks in the DMA-friendly layouts.

    w1b[e]: [FM, P, DK*P]  w1b[fm, ki, ko*P+fi] = w1[e][fm*P+fi, ko*P+ki]
    w2b[e]: [FM, P, HIDDEN] w2b[fk, fi, d]      = w2[e][d, fk*P+fi]
    """
    key = (id(expert_w1), id(expert_w2))
    if key not in _W_CACHE:
        w1b, w2b = [], []
        for e in range(NUM_EXPERTS):
            a = np.asarray(expert_w1[e], dtype=npbf16)          # [FFN, HIDDEN]
            a = a.reshape(FM, P, DK, P).transpose(0, 3, 2, 1)   # fm, ki, ko, fi
            w1b.append(np.ascontiguousarray(a.reshape(FM, P, DK * P)))
            b = np.asarray(expert_w2[e].T, dtype=npbf16)        # [FFN, HIDDEN]
            w2b.append(np.ascontiguousarray(b.reshape(FM, P, HIDDEN)))
        # hold refs so ids stay unique while cached
        _W_CACHE[key] = (w1b, w2b, expert_w1, expert_w2)
    return _W_CACHE[key][:2]


def kernel(x, router_w, expert_w1, expert_w2):
    xf = np.ascontiguousarray(x.reshape(T, HIDDEN), dtype=np.float32)
    xT = np.ascontiguousarray(xf.T)                       # [D, T]
    rT = np.ascontiguousarray(router_w.T.astype(np.float32))  # [D, E]

    # ---- phase A: router on device (data-parallel over tokens) ----
    nc_a = _phase_a_nc()
    in_a = [{"xt": np.ascontiguousarray(xT[:, i * TPC:(i + 1) * TPC]), "rt": rT}
            for i in range(NCORES)]
    res_a = _run_spmd(nc_a, in_a)
    w_all = np.concatenate([res_a.results[i]["w"] for i in range(NCORES)], axis=0)

    # ---- host dispatch: gather each expert's tokens (device-decided) ----
    idx = [np.nonzero(w_all[:, e] > 0.0)[0] for e in range(NUM_EXPERTS)]
    cmax = max(len(i) for i in idx)
    C = min(max(P, ((cmax + P - 1) // P) * P), MAXC)
    n_chunks = (cmax + C - 1) // C
    # bf16 tokens in the (ki, ko, c) DMA layout
    xTb = np.asarray(xT, dtype=npbf16)                    # [D, T]
    x_kic = np.ascontiguousarray(xTb.reshape(DK, P, T).transpose(1, 0, 2))
    w1b, w2b = _expert_weights_bf16(expert_w1, expert_w2)
    nc_b = _phase_b_nc(C)

    out = np.zeros((T, HIDDEN), dtype=np.float32)
    for r in range(n_chunks):
        in_b = []
        for e in range(NUM_EXPERTS):
            ids = idx[e][r * C:(r + 1) * C]
            n = len(ids)
            xsel = np.zeros((P, DK, C), dtype=npbf16)
            xsel[:, :, :n] = x_kic[:, :, ids]
            wsel = np.zeros(C, dtype=np.float32)
            wsel[:n] = w_all[ids, e]
            in_b.append({
                "xt": xsel,
                "w1b": w1b[e],
                "w2b": w2b[e],
                "ws": np.ascontiguousarray(wsel.reshape(C // P, P).T),
            })
        # ---- phase B: expert FFN on device (expert-parallel) ----
        res_b = _run_spmd(nc_b, in_b)
        # ---- host combine: scatter-add (indices within an expert unique) ----
        for e in range(NUM_EXPERTS):
            ids = idx[e][r * C:(r + 1) * C]
            out[ids] += np.asarray(res_b.results[e]["o"][:len(ids)], dtype=np.float32)

    # cost-model exec-time estimate (NTFF profiling unavailable on this path)
    global LAST_HW_NS
    try:
        if ("t", C) not in _B_CACHE:
            from concourse.timeline_sim import TimelineSim
            _B_CACHE[("t", C)] = (TimelineSim(nc_a).simulate()
                                  + TimelineSim(nc_b).simulate() * n_chunks)
        LAST_HW_NS = int(_B_CACHE[("t", C)])
    except Exception:  # noqa: BLE001
        pass
    return out.reshape(BATCH, SEQ, HIDDEN)


# revision 20
# speedup vs baseline: 1.0921x; 1.0010x over previous
"""MoE (top-2 of 8 experts, gelu MLP) on 8 TRN2 NeuronCores.

Strategy (expert-parallel, per the sharding hint):
  Phase A (device, data-parallel over tokens): router scores = x @ router_w.T
    in fp32, top-2 via the DVE max8 instruction, softmax-over-2 via the exact
    sigmoid identity. Outputs per-token per-expert combine weights w[T, E]
    (0 for unselected experts).
  Host dispatch ("all-to-all"): gather each expert's selected token columns
    (from the device-computed weights) into a per-core capacity-padded batch;
    this is the sharding step for phase B.
  Phase B (device, expert-parallel): each core runs one expert's FFN
    out = gelu(xsel @ w1[e].T) @ w2[e].T * w[:, None] over its gathered
    tokens. All matmuls in bf16 (full PE rate, ~5e-3 rel err, well under the
    2e-2 gate). Both weight matrices stay resident in SBUF (16.8 MB bf16),
    tokens stream through in 512-wide tiles, and the FFN contraction for
    mm2 accumulates fully in PSUM - no SBUF accumulator, no re-reads of x.
  Host combine: scatter-add the per-expert results back (each token appears
    in exactly two experts' batches).

kernel(**inputs) -> np.ndarray  takes FULL inputs, returns FULL output.
"""

import numpy as np
import ml_dtypes

import concourse.bass as bass
import concourse.mybir as mybir
from concourse import bacc
from concourse.tile import TileContext
from concourse.bass_utils import run_bass_kernel_spmd

HIDDEN = 1024
NUM_EXPERTS = 8
TOP_K = 2
FFN = 4096
BATCH, SEQ = 4, 2048
T = BATCH * SEQ          # 8192 tokens
NCORES = 8
TPC = T // NCORES        # tokens per core in phase A
P = 128
DK = HIDDEN // P         # 8 contraction tiles over hidden
FM = FFN // P            # 32 f-subtiles (128 wide) over the FFN dim
DN = HIDDEN // 512       # 2 output column blocks
MAXC = 8192              # per-launch token capacity (SBUF-independent here)
WARMUP_MM = 8            # PE p-state warmup matmuls at phase-B start

f32 = mybir.dt.float32
bf16 = mybir.dt.bfloat16
npbf16 = ml_dtypes.bfloat16


def _build_phase_a(repeat=False):
    """Per core: scores for TPC tokens (fp32 matmul) -> top-2 -> weights.

    inputs:  xt [HIDDEN, TPC] fp32 (column shard of x.T), rt [HIDDEN, E] fp32
    output:  w  [TPC, E] fp32 combine weights (0 where expert unselected)
    """
    nc = bacc.Bacc(None)
    xt_d = nc.declare_dram_parameter("xt", [HIDDEN, TPC], f32, isOutput=False)
    rt_d = nc.declare_dram_parameter("rt", [HIDDEN, NUM_EXPERTS], f32, isOutput=False)
    w_d = nc.declare_dram_parameter("w", [TPC, NUM_EXPERTS], f32, isOutput=True)
    if repeat:
        r_d = nc.declare_dram_parameter("r", [1, 1], mybir.dt.uint32, isOutput=False)

    MT = TPC // P  # token tiles per core
    # input DMA chunking (in token tiles): bigger chunks up front for DMA
    # efficiency, single-tile chunks at the end so the last tile's top-2
    # pipeline starts as soon as possible
    chunks = [(0, 2), (2, 2), (4, 2), (6, 1), (7, 1)]
    from contextlib import ExitStack
    with TileContext(nc) as tc, ExitStack() as stk:
        if repeat:
            rp = stk.enter_context(tc.tile_pool(name="rp", bufs=1))
            r_t = rp.tile([1, 1], mybir.dt.uint32)
            nc.sync.dma_start(out=r_t[:], in_=r_d[:])
            _, (r_val,) = nc.values_load_multi_w_load_instructions(
                r_t[:], min_val=1, max_val=1 << 24)
            stk.enter_context(tc.For_i(0, r_val))
        with tc.tile_pool(name="sb", bufs=1) as pool, \
             tc.tile_pool(name="work", bufs=3) as wp, \
             tc.tile_pool(name="ps", bufs=4, space="PSUM") as psp:
            xt_t = pool.tile([P, DK, TPC], f32)
            rt_t = pool.tile([P, DK, NUM_EXPERTS], f32)
            wout = pool.tile([P, MT, NUM_EXPERTS], f32)
            nc.sync.dma_start(out=rt_t[:], in_=rt_d[:].rearrange("(ko ki) e -> ki ko e", ki=P))
            xt_r = xt_d[:].rearrange("(ko ki) t -> ki ko t", ki=P)
            # interleave input chunks across the two HWDGE queues (SP, Act)
            for c, (m, mn) in enumerate(chunks):
                q = nc.sync if c % 2 == 0 else nc.scalar
                q.dma_start(out=xt_t[:, :, m * P:(m + mn) * P],
                            in_=xt_r[:, :, m * P:(m + mn) * P])

            for m in range(MT):
                ps = psp.tile([P, NUM_EXPERTS], f32)
                for k in range(DK):
                    nc.tensor.matmul(ps[:], xt_t[:, k, m * P:(m + 1) * P], rt_t[:, k],
                                     start=(k == 0), stop=(k == DK - 1))
                v8 = wp.tile([P, 8], f32, tag="v8")
                nc.vector.max(out=v8[:], in_=ps[:])
                sg1 = wp.tile([P, 1], f32, tag="sg1")
                sg2 = wp.tile([P, 1], f32, tag="sg2")
                # softmax over top-2 == sigmoid(+/-(v1 - v2)), fused sub via
                # the activation bias/scale path
                nc.scalar.activation(sg1[:], v8[:, 1:2],
                                     mybir.ActivationFunctionType.Sigmoid,
                                     scale=-1.0, bias=v8[:, 0:1])
                nc.scalar.activation(sg2[:], v8[:, 0:1],
                                     mybir.ActivationFunctionType.Sigmoid,
                                     scale=-1.0, bias=v8[:, 1:2])
                m1 = wp.tile([P, NUM_EXPERTS], f32, tag="m1")
                m2 = wp.tile([P, NUM_EXPERTS], f32, tag="m2")
                # w_e = (s==v1)*sg1 + (s==v2)*sg2, two fused compare-mults
                nc.vector.scalar_tensor_tensor(
                    out=m1[:], in0=ps[:], scalar=v8[:, 0:1],
                    in1=sg1[:].to_broadcast([P, NUM_EXPERTS]),
                    op0=mybir.AluOpType.is_equal, op1=mybir.AluOpType.mult)
                nc.vector.scalar_tensor_tensor(
                    out=m2[:], in0=ps[:], scalar=v8[:, 1:2],
                    in1=sg2[:].to_broadcast([P, NUM_EXPERTS]),
                    op0=mybir.AluOpType.is_equal, op1=mybir.AluOpType.mult)
                nc.vector.tensor_add(wout[:, m], m1[:], m2[:])
            # results go out in two DMAs: tiles 0-6 as soon as they are
            # done (overlaps tile 7's pipeline), tile 7 alone in the tail
            w_r = w_d[:].rearrange("(mo p) e -> p mo e", p=P)
            nc.sync.dma_start(out=w_r[:, :MT - 1], in_=wout[:, :MT - 1])
            nc.sync.dma_start(out=w_r[:, MT - 1:], in_=wout[:, MT - 1:])
    nc.compile()
    return nc


def _build_phase_b(C: int, act=None, repeat=False):
    """Per core: one expert's FFN over C gathered tokens (bf16 matmuls).

    inputs: xt  [P, DK, C] bf16     gathered tokens, (ki, ko, c) layout
            w1b [FM, P, DK*P] bf16  expert w1, (fm, ki, ko*fi) layout
            w2b [FM, P, HIDDEN] bf16 expert w2.T, (fk, fi, d) layout
            ws  [P, C//P] f32       per-token combine weight, ws[p, m] = w[m*P+p]
    output: o   [C, HIDDEN] bf16    w * (gelu(x @ w1.T) @ w2.T)
    """
    assert C % P == 0
    if act is None:
        act = mybir.ActivationFunctionType.Gelu
    nc = bacc.Bacc(None)
    xt_d = nc.declare_dram_parameter("xt", [P, DK, C], bf16, isOutput=False)
    w1_d = nc.declare_dram_parameter("w1b", [FM, P, DK * P], bf16, isOutput=False)
    w2_d = nc.declare_dram_parameter("w2b", [FM, P, HIDDEN], bf16, isOutput=False)
    ws_d = nc.declare_dram_parameter("ws", [P, C // P], f32, isOutput=False)
    if repeat:
        r_d = nc.declare_dram_parameter("r", [1, 1], mybir.dt.uint32, isOutput=False)
    o_d = nc.declare_dram_parameter("o", [C, HIDDEN], bf16, isOutput=True)

    # Token tiles: a 256-wide head tile starts PE after only ~1.5us of x DMA
    # (its mm2 is deferred past the next tile's mm1 so the short gelu chain
    # never stalls PE), then 512-wide tiles whose 1.7us/block mm1 pace hides
    # the weight stream, then the 128-multiple remainder.
    if C <= 512:
        t_sizes = [C]
    else:
        rem = (C - 256) % 512
        t_sizes = [256] + [512] * ((C - 256) // 512) + ([rem] if rem else [])
    t_offs = np.cumsum([0] + t_sizes)[:-1].tolist()

    from contextlib import ExitStack
    with TileContext(nc) as tc, ExitStack() as stk:
        if repeat:
            rp = stk.enter_context(tc.tile_pool(name="rp", bufs=1))
            r_t = rp.tile([1, 1], mybir.dt.uint32)
            nc.sync.dma_start(out=r_t[:], in_=r_d[:])
            _, (r_val,) = nc.values_load_multi_w_load_instructions(
                r_t[:], min_val=1, max_val=1 << 24)
            stk.enter_context(tc.For_i(0, r_val))
        with tc.tile_pool(name="wt", bufs=1) as wtp, \
             tc.tile_pool(name="xs", bufs=2) as xsp, \
             tc.tile_pool(name="h", bufs=1) as hp, \
             tc.tile_pool(name="ot", bufs=2) as otp, \
             tc.tile_pool(name="wm", bufs=1, space="PSUM") as wmp, \
             tc.tile_pool(name="ps1", bufs=2, space="PSUM") as ps1p, \
             tc.tile_pool(name="ps2", bufs=2, space="PSUM") as ps2p:
            # dependency-free warmup matmuls: ramp the PE to full p-state
            # during the initial DMA window it would otherwise idle through
            scr = wtp.tile([P, 512], bf16, name="scr")
            nc.vector.memset(scr[:], 0.0)
            wm_ps = wmp.tile([P, 512], f32, name="wm_ps")
            for _ in range(WARMUP_MM):
                nc.tensor.matmul(wm_ps[:], scr[:, :P], scr[:], start=True, stop=True)
            w1s = wtp.tile([P, FM, DK * P], bf16)
            w2s = wtp.tile([P, FM, HIDDEN], bf16)
            ws_t = wtp.tile([P, C // P], f32)
            defer0 = len(t_sizes) > 1
            h0 = wtp.tile([P, FM, t_sizes[0]], bf16, name="h0") if defer0 else None

            # The DMA engines drain transfers in ready order, so x0/x1 are
            # interleaved into the SP w1 stream exactly where mm1(t0)'s
            # 0.85us/block consumption has built enough slack over the
            # 0.73us/block arrivals. Later x tiles are gated by buffer reuse
            # (their transfer can't start before the slot frees), so they
            # never cut ahead of the weight stream.
            x_ts = []
            for ti, (t_off, t_size) in enumerate(zip(t_offs, t_sizes)):
                if ti < 2:
                    x_t = xsp.tile([P, DK, 512], bf16, tag="x",
                                   name="x_t")[:, :, :t_size]
                    x_ts.append(x_t)
                else:
                    x_ts.append(None)  # allocated in the loop (double-buffered)
            for fm in range(FM):
                nc.sync.dma_start(out=w1s[:, fm], in_=w1_d[fm])
                if fm == 0:
                    nc.sync.dma_start(out=x_ts[0][:],
                                      in_=xt_d[:, :, :t_sizes[0]])
                elif fm == 12 and len(t_sizes) > 1:
                    nc.sync.dma_start(out=x_ts[1][:],
                                      in_=xt_d[:, :, t_offs[1]:t_offs[1] + t_sizes[1]])
            for fk in range(FM):
                nc.sync.dma_start(out=w2s[:, fk], in_=w2_d[fk])
            nc.sync.dma_start(out=ws_t[:], in_=ws_d[:])

            o_r = o_d[:].rearrange("(mo p) d -> p mo d", p=P)

            def mm1(x_t, h_t, t_size):
                for fm in range(FM):
                    ph = ps1p.tile([P, 512], f32, tag="ph", name="ph")[:, :t_size]
                    for k in range(DK):
                        nc.tensor.matmul(ph[:], w1s[:, fm, k * P:(k + 1) * P],
                                         x_t[:, k], start=(k == 0), stop=(k == DK - 1))
                    nc.scalar.activation(h_t[:, fm], ph[:], act)

            def mm2(h_t, t_off, t_size):
                for tm in range(t_size // P):
                    mt = t_off // P + tm
                    for dn in range(DN):
                        po = ps2p.tile([P, 512], f32, tag="po", name="po")
                        for fk in range(FM):
                            nc.tensor.matmul(po[:],
                                             h_t[:, fk, tm * P:(tm + 1) * P],
                                             w2s[:, fk, dn * 512:(dn + 1) * 512],
                                             start=(fk == 0), stop=(fk == FM - 1))
                        o_t = otp.tile([P, 512], bf16, tag="o", name="o_t")
                        nc.vector.tensor_scalar_mul(o_t[:], po[:], ws_t[:, mt:mt + 1])
                        nc.sync.dma_start(out=o_r[:, mt, dn * 512:(dn + 1) * 512],
                                          in_=o_t[:])

            for ti, (t_off, t_size) in enumerate(zip(t_offs, t_sizes)):
                x_t = x_ts[ti]
                if x_t is None:
                    x_t = xsp.tile([P, DK, 512], bf16, tag="x",
                                   name="x_t")[:, :, :t_size]
                    nc.scalar.dma_start(out=x_t[:],
                                        in_=xt_d[:, :, t_off:t_off + t_size])
                if defer0 and ti == 0:
                    mm1(x_t, h0, t_size)
                    continue
                h_t = hp.tile([P, FM, 512], bf16, tag="h", name="h_t")[:, :, :t_size]
                mm1(x_t, h_t, t_size)
                if defer0 and ti == 1:
                    mm2(h0, 0, t_sizes[0])
                mm2(h_t, t_off, t_size)
    nc.compile()
    return nc


_A_CACHE = {}
_B_CACHE = {}
_W_CACHE = {}
LAST_HW_NS = None


def _run_spmd(nc, in_maps, retries=2):
    """run_bass_kernel_spmd with retry: device crashes on this axon path are
    occasionally transient (NRT_EXEC_UNIT_UNRECOVERABLE recovers on a fresh
    attempt)."""
    last = None
    for attempt in range(retries + 1):
        try:
            return run_bass_kernel_spmd(nc, in_maps, list(range(NCORES)))
        except Exception as e:  # noqa: BLE001
            last = e
            import time as _time
            _time.sleep(2.0 * (attempt + 1))
    raise last


def _phase_a_nc():
    if "a" not in _A_CACHE:
        _A_CACHE["a"] = _build_phase_a()
    return _A_CACHE["a"]


def _phase_b_nc(C):
    if C not in _B_CACHE:
        _B_CACHE[C] = _build_phase_b(C)
    return _B_CACHE[C]


def _expert_weights_bf16(expert_w1, expert_w2):
    """Per-expert bf16 weight blocks in the DMA-friendly layouts.

    w1b[e]: [FM, P, DK*P]  w1b[fm, ki, ko*P+fi] = w1[e][fm*P+fi, ko*P+ki]
    w2b[e]: [FM, P, HIDDEN] w2b[fk, fi, d]      = w2[e][d, fk*P+fi]
    """
    key = (id(expert_w1), id(expert_w2))
    if key not in _W_CACHE:
        w1b, w2b = [], []
        for e in range(NUM_EXPERTS):
            a = np.asarray(expert_w1[e], dtype=npbf16)          # [FFN, HIDDEN]
            a = a.reshape(FM, P, DK, P).transpose(0, 3, 2, 1)   # fm, ki, ko, fi
            w1b.append(np.ascontiguousarray(a.reshape(FM, P, DK * P)))
            b = np.asarray(expert_w2[e].T, dtype=npbf16)        # [FFN, HIDDEN]
            w2b.append(np.ascontiguousarray(b.reshape(FM, P, HIDDEN)))
        # hold refs so ids stay unique while cached
        _W_CACHE[key] = (w1b, w2b, expert_w1, expert_w2)
    return _W_CACHE[key][:2]


def kernel(x, router_w, expert_w1, expert_w2):
    xf = np.ascontiguousarray(x.reshape(T, HIDDEN), dtype=np.float32)
    xT = np.ascontiguousarray(xf.T)                       # [D, T]
    rT = np.ascontiguousarray(router_w.T.astype(np.float32))  # [D, E]

    # ---- phase A: router on device (data-parallel over tokens) ----
    nc_a = _phase_a_nc()
    in_a = [{"xt": np.ascontiguousarray(xT[:, i * TPC:(i + 1) * TPC]), "rt": rT}
            for i in range(NCORES)]
    res_a = _run_spmd(nc_a, in_a)
    w_all = np.concatenate([res_a.results[i]["w"] for i in range(NCORES)], axis=0)

    # ---- host dispatch: gather each expert's tokens (device-decided) ----
    idx = [np.nonzero(w_all[:, e] > 0.0)[0] for e in range(NUM_EXPERTS)]
    cmax = max(len(i) for i in idx)
    C = min(max(P, ((cmax + P - 1) // P) * P), MAXC)
    n_chunks = (cmax + C - 1) // C
    # bf16 tokens in the (ki, ko, c) DMA layout
    xTb = np.asarray(xT, dtype=npbf16)                    # [D, T]
    x_kic = np.ascontiguousarray(xTb.reshape(DK, P, T).transpose(1, 0, 2))
    w1b, w2b = _expert_weights_bf16(expert_w1, expert_w2)
    nc_b = _phase_b_nc(C)

    out = np.zeros((T, HIDDEN), dtype=np.float32)
    for r in range(n_chunks):
        in_b = []
        for e in range(NUM_EXPERTS):
            ids = idx[e][r * C:(r + 1) * C]
            n = len(ids)
            xsel = np.zeros((P, DK, C), dtype=npbf16)
            xsel[:, :, :n] = x_kic[:, :, ids]
            wsel = np.zeros(C, dtype=np.float32)
            wsel[:n] = w_all[ids, e]
            in_b.append({
                "xt": xsel,
                "w1b": w1b[e],
                "w2b": w2b[e],
                "ws": np.ascontiguousarray(wsel.reshape(C // P, P).T),
            })
        # ---- phase B: expert FFN on device (expert-parallel) ----
        res_b = _run_spmd(nc_b, in_b)
        # ---- host combine: scatter-add (indices within an expert unique) ----
        for e in range(NUM_EXPERTS):
            ids = idx[e][r * C:(r + 1) * C]
            out[ids] += np.asarray(res_b.results[e]["o"][:len(ids)], dtype=np.float32)

    # cost-model exec-time estimate (NTFF profiling unavailable on this path)
    global LAST_HW_NS
    try:
        if ("t", C) not in _B_CACHE:
            from concourse.timeline_sim import TimelineSim
            _B_CACHE[("t", C)] = (TimelineSim(nc_a).simulate()
                                  + TimelineSim(nc_b).simulate() * n_chunks)
        LAST_HW_NS = int(_B_CACHE[("t", C)])
    except Exception:  # noqa: BLE001
        pass
    return out.reshape(BATCH, SEQ, HIDDEN)


# revision 26
# speedup vs baseline: 1.1045x; 1.0114x over previous
"""MoE (top-2 of 8 experts, gelu MLP) on 8 TRN2 NeuronCores.

Strategy (expert-parallel, per the sharding hint):
  Phase A (device, data-parallel over tokens): router scores = x @ router_w.T
    in bf16 (halves the DMA that dominates this phase), top-2 via the DVE
    max8 instruction, softmax-over-2 via the exact sigmoid identity. Outputs
    per-token per-expert combine weights w[T, E] (0 for unselected experts)
    plus the top2/top3 margin; the ~1% of tokens within bf16 noise of a
    selection tie are re-routed exactly on the host (0.016% of model FLOPs).
  Host dispatch ("all-to-all"): gather each expert's selected token columns
    (from the device-computed weights) into a per-core capacity-padded batch;
    this is the sharding step for phase B.
  Phase B (device, expert-parallel): each core runs one expert's FFN
    out = gelu(xsel @ w1[e].T) @ w2[e].T * w[:, None] over its gathered
    tokens. All matmuls in bf16 (full PE rate, ~5e-3 rel err, well under the
    2e-2 gate). Both weight matrices stay resident in SBUF (16.8 MB bf16),
    tokens stream through in 512-wide tiles, and the FFN contraction for
    mm2 accumulates fully in PSUM - no SBUF accumulator, no re-reads of x.
  Host combine: scatter-add the per-expert results back (each token appears
    in exactly two experts' batches).

kernel(**inputs) -> np.ndarray  takes FULL inputs, returns FULL output.
"""

import numpy as np
import ml_dtypes

import concourse.bass as bass
import concourse.mybir as mybir
from concourse import bacc
from concourse.tile import TileContext
from concourse.bass_utils import run_bass_kernel_spmd

HIDDEN = 1024
NUM_EXPERTS = 8
TOP_K = 2
FFN = 4096
BATCH, SEQ = 4, 2048
T = BATCH * SEQ          # 8192 tokens
NCORES = 8
TPC = T // NCORES        # tokens per core in phase A
P = 128
DK = HIDDEN // P         # 8 contraction tiles over hidden
FM = FFN // P            # 32 f-subtiles (128 wide) over the FFN dim
DN = HIDDEN // 512       # 2 output column blocks
MAXC = 8192              # per-launch token capacity (SBUF-independent here)
WARMUP_MM = 4            # PE p-state warmup matmuls at phase-B start

f32 = mybir.dt.float32
bf16 = mybir.dt.bfloat16
npbf16 = ml_dtypes.bfloat16


def _build_phase_a(repeat=False):
    """Per core: scores for TPC tokens (bf16 matmul, f32 accumulate) -> top-2
    -> weights, plus the top2/top3 margin so the host can exactly re-route
    the ~1% of tokens whose selection is within bf16 noise of a tie.

    inputs:  xt [HIDDEN, TPC] bf16 (column shard of x.T), rt [HIDDEN, E] bf16
    output:  w  [TPC, E+1]: combine weights (0 where expert unselected),
             last column = v2 - v3 margin
    """
    nc = bacc.Bacc(None)
    xt_d = nc.declare_dram_parameter("xt", [HIDDEN, TPC], bf16, isOutput=False)
    rt_d = nc.declare_dram_parameter("rt", [HIDDEN, NUM_EXPERTS], bf16, isOutput=False)
    w_d = nc.declare_dram_parameter("w", [TPC, NUM_EXPERTS + 1], f32, isOutput=True)
    if repeat:
        r_d = nc.declare_dram_parameter("r", [1, 1], mybir.dt.uint32, isOutput=False)

    MT = TPC // P  # token tiles per core
    # input DMA chunking (in token tiles): bigger chunks up front for DMA
    # efficiency, single-tile chunks at the end so the last tile's top-2
    # pipeline starts as soon as possible
    chunks = [(0, 2), (2, 2), (4, 2), (6, 1), (7, 1)]
    from contextlib import ExitStack
    with TileContext(nc) as tc, ExitStack() as stk:
        if repeat:
            rp = stk.enter_context(tc.tile_pool(name="rp", bufs=1))
            r_t = rp.tile([1, 1], mybir.dt.uint32)
            nc.sync.dma_start(out=r_t[:], in_=r_d[:])
            _, (r_val,) = nc.values_load_multi_w_load_instructions(
                r_t[:], min_val=1, max_val=1 << 24)
            stk.enter_context(tc.For_i(0, r_val))
        with tc.tile_pool(name="sb", bufs=1) as pool, \
             tc.tile_pool(name="work", bufs=3) as wp, \
             tc.tile_pool(name="ps", bufs=4, space="PSUM") as psp:
            xt_t = pool.tile([P, DK, TPC], bf16)
            rt_t = pool.tile([P, DK, NUM_EXPERTS], bf16)
            wout = pool.tile([P, MT, NUM_EXPERTS + 1], f32)
            nc.sync.dma_start(out=rt_t[:], in_=rt_d[:].rearrange("(ko ki) e -> ki ko e", ki=P))
            xt_r = xt_d[:].rearrange("(ko ki) t -> ki ko t", ki=P)
            # interleave input chunks across the two HWDGE queues (SP, Act)
            for c, (m, mn) in enumerate(chunks):
                q = nc.sync if c % 2 == 0 else nc.scalar
                q.dma_start(out=xt_t[:, :, m * P:(m + mn) * P],
                            in_=xt_r[:, :, m * P:(m + mn) * P])

            for m in range(MT):
                ps = psp.tile([P, NUM_EXPERTS], f32)
                for k in range(DK):
                    nc.tensor.matmul(ps[:], xt_t[:, k, m * P:(m + 1) * P], rt_t[:, k],
                                     start=(k == 0), stop=(k == DK - 1))
                v8 = wp.tile([P, 8], f32, tag="v8")
                nc.vector.max(out=v8[:], in_=ps[:])
                sg1 = wp.tile([P, 1], f32, tag="sg1")
                sg2 = wp.tile([P, 1], f32, tag="sg2")
                # softmax over top-2 == sigmoid(+/-(v1 - v2)), fused sub via
                # the activation bias/scale path
                nc.scalar.activation(sg1[:], v8[:, 1:2],
                                     mybir.ActivationFunctionType.Sigmoid,
                                     scale=-1.0, bias=v8[:, 0:1])
                nc.scalar.activation(sg2[:], v8[:, 0:1],
                                     mybir.ActivationFunctionType.Sigmoid,
                                     scale=-1.0, bias=v8[:, 1:2])
                m1 = wp.tile([P, NUM_EXPERTS], f32, tag="m1")
                m2 = wp.tile([P, NUM_EXPERTS], f32, tag="m2")
                # w_e = (s==v1)*sg1 + (s==v2)*sg2, two fused compare-mults
                nc.vector.scalar_tensor_tensor(
                    out=m1[:], in0=ps[:], scalar=v8[:, 0:1],
                    in1=sg1[:].to_broadcast([P, NUM_EXPERTS]),
                    op0=mybir.AluOpType.is_equal, op1=mybir.AluOpType.mult)
                nc.vector.scalar_tensor_tensor(
                    out=m2[:], in0=ps[:], scalar=v8[:, 1:2],
                    in1=sg2[:].to_broadcast([P, NUM_EXPERTS]),
                    op0=mybir.AluOpType.is_equal, op1=mybir.AluOpType.mult)
                nc.vector.tensor_add(wout[:, m, :NUM_EXPERTS], m1[:], m2[:])
                nc.vector.tensor_sub(wout[:, m, NUM_EXPERTS:], v8[:, 1:2], v8[:, 2:3])
            # results go out in two DMAs: tiles 0-6 as soon as they are
            # done (overlaps tile 7's pipeline), tile 7 alone in the tail
            w_r = w_d[:].rearrange("(mo p) e -> p mo e", p=P)
            nc.sync.dma_start(out=w_r[:, :MT - 1], in_=wout[:, :MT - 1])
            nc.sync.dma_start(out=w_r[:, MT - 1:], in_=wout[:, MT - 1:])
    nc.compile()
    return nc


def _build_phase_b(C: int, act=None, repeat=False):
    """Per core: one expert's FFN over C gathered tokens (bf16 matmuls).

    inputs: xt  [P, DK, C] bf16     gathered tokens, (ki, ko, c) layout
            w1b [FM, P, DK*P] bf16  expert w1, (fm, ki, ko*fi) layout
            w2b [FM, P, HIDDEN] bf16 expert w2.T, (fk, fi, d) layout
            ws  [P, C//P] f32       per-token combine weight, ws[p, m] = w[m*P+p]
    output: o   [C, HIDDEN] bf16    w * (gelu(x @ w1.T) @ w2.T)
    """
    assert C % P == 0
    if act is None:
        act = mybir.ActivationFunctionType.Gelu
    nc = bacc.Bacc(None)
    xt_d = nc.declare_dram_parameter("xt", [P, DK, C], bf16, isOutput=False)
    w1_d = nc.declare_dram_parameter("w1b", [FM, P, DK * P], bf16, isOutput=False)
    w2_d = nc.declare_dram_parameter("w2b", [FM, P, HIDDEN], bf16, isOutput=False)
    ws_d = nc.declare_dram_parameter("ws", [P, C // P], f32, isOutput=False)
    if repeat:
        r_d = nc.declare_dram_parameter("r", [1, 1], mybir.dt.uint32, isOutput=False)
    o_d = nc.declare_dram_parameter("o", [C, HIDDEN], bf16, isOutput=True)

    # Token tiles: a 256-wide head tile starts PE after only ~1.5us of x DMA
    # (its mm2 is deferred past the next tile's mm1 so the short gelu chain
    # never stalls PE), then 512-wide tiles whose 1.7us/block mm1 pace hides
    # the weight stream, then the 128-multiple remainder.
    if C <= 512:
        t_sizes = [C]
    else:
        rem = (C - 256) % 512
        t_sizes = [256] + [512] * ((C - 256) // 512) + ([rem] if rem else [])
    t_offs = np.cumsum([0] + t_sizes)[:-1].tolist()

    from contextlib import ExitStack
    with TileContext(nc) as tc, ExitStack() as stk:
        if repeat:
            rp = stk.enter_context(tc.tile_pool(name="rp", bufs=1))
            r_t = rp.tile([1, 1], mybir.dt.uint32)
            nc.sync.dma_start(out=r_t[:], in_=r_d[:])
            _, (r_val,) = nc.values_load_multi_w_load_instructions(
                r_t[:], min_val=1, max_val=1 << 24)
            stk.enter_context(tc.For_i(0, r_val))
        with tc.tile_pool(name="wt", bufs=1) as wtp, \
             tc.tile_pool(name="xs", bufs=2) as xsp, \
             tc.tile_pool(name="h", bufs=1) as hp, \
             tc.tile_pool(name="ot", bufs=2) as otp, \
             tc.tile_pool(name="wm", bufs=1, space="PSUM") as wmp, \
             tc.tile_pool(name="ps1", bufs=2, space="PSUM") as ps1p, \
             tc.tile_pool(name="ps2", bufs=2, space="PSUM") as ps2p:
            # dependency-free warmup matmuls: ramp the PE to full p-state
            # during the initial DMA window it would otherwise idle through
            scr = wtp.tile([P, 512], bf16, name="scr")
            nc.vector.memset(scr[:], 0.0)
            wm_ps = wmp.tile([P, 512], f32, name="wm_ps")
            for _ in range(WARMUP_MM):
                nc.tensor.matmul(wm_ps[:], scr[:, :P], scr[:], start=True, stop=True)
            w1s = wtp.tile([P, FM, DK * P], bf16)
            w2s = wtp.tile([P, FM, HIDDEN], bf16)
            ws_t = wtp.tile([P, C // P], f32)
            defer0 = len(t_sizes) > 1
            h0 = wtp.tile([P, FM, t_sizes[0]], bf16, name="h0") if defer0 else None

            # The DMA engines drain transfers in ready order, so x0/x1 are
            # interleaved into the SP w1 stream exactly where mm1(t0)'s
            # 0.85us/block consumption has built enough slack over the
            # 0.73us/block arrivals. Later x tiles are gated by buffer reuse
            # (their transfer can't start before the slot frees), so they
            # never cut ahead of the weight stream.
            x_ts = []
            for ti, (t_off, t_size) in enumerate(zip(t_offs, t_sizes)):
                if ti < 2:
                    x_t = xsp.tile([P, DK, 512], bf16, tag="x",
                                   name="x_t")[:, :, :t_size]
                    x_ts.append(x_t)
                else:
                    x_ts.append(None)  # allocated in the loop (double-buffered)
            for fm in range(FM):
                nc.sync.dma_start(out=w1s[:, fm], in_=w1_d[fm])
                if fm == 0:
                    nc.sync.dma_start(out=x_ts[0][:],
                                      in_=xt_d[:, :, :t_sizes[0]])
                elif fm == 12 and len(t_sizes) > 1:
                    nc.sync.dma_start(out=x_ts[1][:],
                                      in_=xt_d[:, :, t_offs[1]:t_offs[1] + t_sizes[1]])
            for fk in range(FM):
                nc.sync.dma_start(out=w2s[:, fk], in_=w2_d[fk])
            nc.sync.dma_start(out=ws_t[:], in_=ws_d[:])

            o_r = o_d[:].rearrange("(mo p) d -> p mo d", p=P)

            def mm1(x_t, h_t, t_size):
                for fm in range(FM):
                    ph = ps1p.tile([P, 512], f32, tag="ph", name="ph")[:, :t_size]
                    for k in range(DK):
                        nc.tensor.matmul(ph[:], w1s[:, fm, k * P:(k + 1) * P],
                                         x_t[:, k], start=(k == 0), stop=(k == DK - 1))
                    nc.scalar.activation(h_t[:, fm], ph[:], act)

            def mm2(h_t, t_off, t_size):
                for tm in range(t_size // P):
                    mt = t_off // P + tm
                    for dn in range(DN):
                        po = ps2p.tile([P, 512], f32, tag="po", name="po")
                        for fk in range(FM):
                            nc.tensor.matmul(po[:],
                                             h_t[:, fk, tm * P:(tm + 1) * P],
                                             w2s[:, fk, dn * 512:(dn + 1) * 512],
                                             start=(fk == 0), stop=(fk == FM - 1))
                        o_t = otp.tile([P, 512], bf16, tag="o", name="o_t")
                        nc.vector.tensor_scalar_mul(o_t[:], po[:], ws_t[:, mt:mt + 1])
                        nc.sync.dma_start(out=o_r[:, mt, dn * 512:(dn + 1) * 512],
                                          in_=o_t[:])

            for ti, (t_off, t_size) in enumerate(zip(t_offs, t_sizes)):
                x_t = x_ts[ti]
                if x_t is None:
                    x_t = xsp.tile([P, DK, 512], bf16, tag="x",
                                   name="x_t")[:, :, :t_size]
                    nc.scalar.dma_start(out=x_t[:],
                                        in_=xt_d[:, :, t_off:t_off + t_size])
                if defer0 and ti == 0:
                    mm1(x_t, h0, t_size)
                    continue
                h_t = hp.tile([P, FM, 512], bf16, tag="h", name="h_t")[:, :, :t_size]
                mm1(x_t, h_t, t_size)
                if defer0 and ti == 1:
                    mm2(h0, 0, t_sizes[0])
                mm2(h_t, t_off, t_size)
    nc.compile()
    return nc


_A_CACHE = {}
_B_CACHE = {}
_W_CACHE = {}
LAST_HW_NS = None


def _run_spmd(nc, in_maps, retries=2):
    """run_bass_kernel_spmd with retry: device crashes on this axon path are
    occasionally transient (NRT_EXEC_UNIT_UNRECOVERABLE recovers on a fresh
    attempt)."""
    last = None
    for attempt in range(retries + 1):
        try:
            return run_bass_kernel_spmd(nc, in_maps, list(range(NCORES)))
        except Exception as e:  # noqa: BLE001
            last = e
            import time as _time
            _time.sleep(2.0 * (attempt + 1))
    raise last


def _phase_a_nc():
    if "a" not in _A_CACHE:
        _A_CACHE["a"] = _build_phase_a()
    return _A_CACHE["a"]


def _phase_b_nc(C):
    if C not in _B_CACHE:
        _B_CACHE[C] = _build_phase_b(C)
    return _B_CACHE[C]


def _expert_weights_bf16(expert_w1, expert_w2):
    """Per-expert bf16 weight blocks in the DMA-friendly layouts.

    w1b[e]: [FM, P, DK*P]  w1b[fm, ki, ko*P+fi] = w1[e][fm*P+fi, ko*P+ki]
    w2b[e]: [FM, P, HIDDEN] w2b[fk, fi, d]      = w2[e][d, fk*P+fi]
    """
    key = (id(expert_w1), id(expert_w2))
    if key not in _W_CACHE:
        w1b, w2b = [], []
        for e in range(NUM_EXPERTS):
            a = np.asarray(expert_w1[e], dtype=npbf16)          # [FFN, HIDDEN]
            a = a.reshape(FM, P, DK, P).transpose(0, 3, 2, 1)   # fm, ki, ko, fi
            w1b.append(np.ascontiguousarray(a.reshape(FM, P, DK * P)))
            b = np.asarray(expert_w2[e].T, dtype=npbf16)        # [FFN, HIDDEN]
            w2b.append(np.ascontiguousarray(b.reshape(FM, P, HIDDEN)))
        # hold refs so ids stay unique while cached
        _W_CACHE[key] = (w1b, w2b, expert_w1, expert_w2)
    return _W_CACHE[key][:2]


def kernel(x, router_w, expert_w1, expert_w2):
    xf = np.ascontiguousarray(x.reshape(T, HIDDEN), dtype=np.float32)
    xT = np.ascontiguousarray(xf.T)                       # [D, T]
    rT = np.ascontiguousarray(router_w.T.astype(np.float32))  # [D, E]
    xTb = np.asarray(xT, dtype=npbf16)                    # [D, T] bf16
    rTb = np.asarray(rT, dtype=npbf16)

    # ---- phase A: router on device (data-parallel over tokens) ----
    nc_a = _phase_a_nc()
    in_a = [{"xt": np.ascontiguousarray(xTb[:, i * TPC:(i + 1) * TPC]), "rt": rTb}
            for i in range(NCORES)]
    res_a = _run_spmd(nc_a, in_a)
    w_all9 = np.concatenate([res_a.results[i]["w"] for i in range(NCORES)], axis=0)
    w_all = np.ascontiguousarray(w_all9[:, :NUM_EXPERTS])
    marg = w_all9[:, NUM_EXPERTS]

    # Tokens whose top-2 selection sits within bf16 noise of a tie (small
    # v2-v3 margin, near-0.5 top weight, or a double-matched exact tie) get
    # their routing redone exactly; ~1% of tokens, 0.016% of model FLOPs.
    TAU = 0.03
    smax = w_all.max(axis=1)
    ssum = w_all.sum(axis=1)
    flag = (marg < TAU) | (smax < 0.508) | (np.abs(ssum - 1.0) > 1e-3)
    ids_f = np.nonzero(flag)[0]
    if len(ids_f):
        s = xf[ids_f].astype(np.float64) @ rT.astype(np.float64)   # [n, E]
        i2 = np.argsort(-s, axis=1)[:, :TOP_K]
        v = np.take_along_axis(s, i2, axis=1)
        sg = 1.0 / (1.0 + np.exp(-(v[:, 0] - v[:, 1])))
        wrows = np.zeros((len(ids_f), NUM_EXPERTS))
        np.put_along_axis(wrows, i2[:, 0:1], sg[:, None], axis=1)
        np.put_along_axis(wrows, i2[:, 1:2], 1.0 - sg[:, None], axis=1)
        w_all[ids_f] = wrows.astype(np.float32)

    # ---- host dispatch: gather each expert's tokens (device-decided) ----
    idx = [np.nonzero(w_all[:, e] > 0.0)[0] for e in range(NUM_EXPERTS)]
    cmax = max(len(i) for i in idx)
    C = min(max(P, ((cmax + P - 1) // P) * P), MAXC)
    n_chunks = (cmax + C - 1) // C
    # bf16 tokens in the (ki, ko, c) DMA layout
    xTb = np.asarray(xT, dtype=npbf16)                    # [D, T]
    x_kic = np.ascontiguousarray(xTb.reshape(DK, P, T).transpose(1, 0, 2))
    w1b, w2b = _expert_weights_bf16(expert_w1, expert_w2)
    nc_b = _phase_b_nc(C)

    out = np.zeros((T, HIDDEN), dtype=np.float32)
    for r in range(n_chunks):
        in_b = []
        for e in range(NUM_EXPERTS):
            ids = idx[e][r * C:(r + 1) * C]
            n = len(ids)
            xsel = np.zeros((P, DK, C), dtype=npbf16)
            xsel[:, :, :n] = x_kic[:, :, ids]
            wsel = np.zeros(C, dtype=np.float32)
            wsel[:n] = w_all[ids, e]
            in_b.append({
                "xt": xsel,
                "w1b": w1b[e],
                "w2b": w2b[e],
                "ws": np.ascontiguousarray(wsel.reshape(C // P, P).T),
            })
        # ---- phase B: expert FFN on device (expert-parallel) ----
        res_b = _run_spmd(nc_b, in_b)
        # ---- host combine: scatter-add (indices within an expert unique) ----
        for e in range(NUM_EXPERTS):
            ids = idx[e][r * C:(r + 1) * C]
            out[ids] += np.asarray(res_b.results[e]["o"][:len(ids)], dtype=np.float32)

    # cost-model exec-time estimate (NTFF profiling unavailable on this path)
    global LAST_HW_NS
    try:
        if ("t", C) not in _B_CACHE:
            from concourse.timeline_sim import TimelineSim
            _B_CACHE[("t", C)] = (TimelineSim(nc_a).simulate()
                                  + TimelineSim(nc_b).simulate() * n_chunks)
        LAST_HW_NS = int(_B_CACHE[("t", C)])
    except Exception:  # noqa: BLE001
        pass
    return out.reshape(BATCH, SEQ, HIDDEN)
